# revision 7
# baseline (speedup 1.0000x reference)
"""Trainium2 Bass kernel for nn_EquivariantDecoder.

Data-parallel over 8 NeuronCores (batch sharded, 2048 rows/core).

Host side packs v_raw into a feature-major bf16 layout (one contiguous
[128, 30*BT] block per b-tile) so the device does zero transposes and
zero casting DMAs: per b-tile ONE 1.9MB HWDGE load, then the four
e3linear layers run as weight-stationary bf16 matmuls whose moving
operand is the batch dimension. Per-(l,m) blocks are packed two-deep in
the contraction dim (l3|l4, l5|l6, l1|l2) so most matmuls use the full
128 partitions. Gates (sigmoid/silu) run on ACT, gating multiplies on
DVE, PSUM z-tiles cycle through 2x3-bank buffers. The final layer
accumulates all 49 outputs into one PSUM bank; results are stored
feature-major [49, BC] and transposed back on the host.
"""

import numpy as np
import ml_dtypes
from contextlib import ExitStack

import concourse.bass as bass
import concourse.mybir as mybir
import concourse.tile as tile
from concourse import bass_utils

BF16 = mybir.dt.bfloat16
FP32 = mybir.dt.float32
BF = ml_dtypes.bfloat16

# ---------------- problem constants (hardcoded) ----------------
B_FULL = 16384
NCORES = 8
BC = B_FULL // NCORES          # 2048 rows per core
BT = 512                       # b-tile
NT = BC // BT

IN_IRREPS = [(256, 0), (128, 1), (128, 2), (64, 3), (64, 4), (64, 5), (64, 6)]
HID_IRREPS = [(64, 0), (64, 1), (64, 2), (32, 3), (32, 4), (32, 5), (32, 6)]
N_SCALARS = 64
N_GATES = 256
D_IN = 3840
D_OUT = 49

IN_OFF = {}
_o = 0
for _mul, _l in IN_IRREPS:
    IN_OFF[_l] = _o
    _o += _mul * (2 * _l + 1)

OUT_OFF = {l: l * l for l in range(7)}

# gate channel permutation: [g_l2|g_l1 | g_l6|g_l5|g_l4|g_l3]
GPERM = ([64 + i for i in range(64)] + [i for i in range(64)] +
         [224 + i for i in range(32)] + [192 + i for i in range(32)] +
         [160 + i for i in range(32)] + [128 + i for i in range(32)])

_BUILD = {}


def _build_P():
    """Feature permutation: 30 partition-blocks of 128 in device order."""
    P = []
    P += list(range(0, 256))                                   # g0,g1: l0
    for m in range(3):                                         # g2..4: l1
        P += [IN_OFF[1] + i * 3 + m for i in range(128)]
    for m in range(5):                                         # g5..9: l2
        P += [IN_OFF[2] + i * 5 + m for i in range(128)]
    for m in range(7):                                         # g10..16: l3|l4
        P += [IN_OFF[3] + i * 7 + m for i in range(64)]
        P += [IN_OFF[4] + i * 9 + m for i in range(64)]
    P += [IN_OFF[4] + i * 9 + 7 for i in range(64)]            # g17: l4 m7|m8
    P += [IN_OFF[4] + i * 9 + 8 for i in range(64)]
    for m in range(11):                                        # g18..28: l5|l6
        P += [IN_OFF[5] + i * 11 + m for i in range(64)]
        P += [IN_OFF[6] + i * 13 + m for i in range(64)]
    P += [IN_OFF[6] + i * 13 + 11 for i in range(64)]          # g29: l6 m11|m12
    P += [IN_OFF[6] + i * 13 + 12 for i in range(64)]
    return np.array(P, np.int64)


P_FEAT = _build_P()


def _split_blocks(wflat, in_irr, out_irr):
    mul_in = {l: m for m, l in in_irr}
    blocks = []
    off = 0
    for mo, l in out_irr:
        mi = mul_in[l]
        w = wflat[off:off + mi * mo].reshape(mi, mo) / np.sqrt(mi)
        off += mi * mo
        blocks.append((l, w))
    assert off == wflat.size
    return blocks


def _pack_weights(w1, w2, w3, w4):
    out = {}
    pre = [(N_SCALARS, 0), (N_GATES, 0)] + [(m, l) for m, l in HID_IRREPS if l > 0]

    b1 = _split_blocks(w1, IN_IRREPS, pre)
    ws, wg = b1[0][1], b1[1][1]
    W10 = np.concatenate([ws, wg[:, GPERM]], axis=1)           # [256, 320]
    out["W1_0a"], out["W1_0b"] = W10[:128].astype(BF), W10[128:].astype(BF)
    wl = {l: w for l, w in b1[2:]}
    out["W1_l1"] = wl[1].astype(BF)                            # [128, 64]
    out["W1_l2"] = wl[2].astype(BF)                            # [128, 64]
    W134 = np.zeros((128, 64), np.float32)
    W134[0:64, 32:64] = wl[3]      # l3 -> psum 96:128 (out base 64)
    W134[64:128, 0:32] = wl[4]     # l4 -> psum 64:96
    out["W1_34"] = W134.astype(BF)
    # duplicated across both partition halves: matmul requires stationary
    # and moving operands to start at the same partition index
    out["W1_l4"] = np.concatenate([wl[4], wl[4]], axis=0).astype(BF)  # [128, 32]
    W156 = np.zeros((128, 64), np.float32)
    W156[0:64, 32:64] = wl[5]      # l5 -> psum 32:64
    W156[64:128, 0:32] = wl[6]     # l6 -> psum 0:32
    out["W1_56"] = W156.astype(BF)
    out["W1_l6"] = np.concatenate([wl[6], wl[6]], axis=0).astype(BF)  # [128, 32]

    for name, wflat in (("W2", w2), ("W3", w3)):
        b = _split_blocks(wflat, HID_IRREPS, pre)
        ws, wg = b[0][1], b[1][1]
        out[name + "_0"] = np.concatenate([ws, wg[:, GPERM]], axis=1).astype(BF)
        wl = {l: w for l, w in b[2:]}
        W12 = np.zeros((128, 128), np.float32)
        W12[0:64, 0:64] = wl[2]
        W12[64:128, 64:128] = wl[1]
        out[name + "_12"] = W12.astype(BF)
        out[name + "_l2"] = wl[2].astype(BF)                   # [64, 64]
        WB4 = np.zeros((128, 128), np.float32)
        for j, l in enumerate((6, 5, 4, 3)):
            WB4[32 * j:32 * (j + 1), 32 * j:32 * (j + 1)] = wl[l]
        out[name + "_B4"] = WB4.astype(BF)

    b4 = _split_blocks(w4, HID_IRREPS, [(1, l) for l in range(7)])
    w4l = {l: w[:, 0] for l, w in b4}
    W4B = np.zeros((128, 13, D_OUT), np.float32)
    for l in (3, 4, 5, 6):
        pd = 32 * (6 - l)
        for m in range(2 * l + 1):
            W4B[pd:pd + 32, m, OUT_OFF[l] + m] = w4l[l]
    out["W4_B"] = W4B.astype(BF)
    W4A = np.zeros((128, 5, D_OUT), np.float32)
    for m in range(5):
        W4A[0:64, m, OUT_OFF[2] + m] = w4l[2]
    for m in range(3):
        W4A[64:128, m, OUT_OFF[1] + m] = w4l[1]
    out["W4_A"] = W4A.astype(BF)
    W40 = np.zeros((64, D_OUT), np.float32)
    W40[:, 0] = w4l[0]
    out["W4_0"] = W40.astype(BF)
    return out


_WSPECS = [
    ("W1_0a", [128, 320]), ("W1_0b", [128, 320]),
    ("W1_l1", [128, 64]), ("W1_l2", [128, 64]),
    ("W1_34", [128, 64]), ("W1_l4", [128, 32]),
    ("W1_56", [128, 64]), ("W1_l6", [128, 32]),
    ("W2_0", [64, 320]), ("W2_12", [128, 128]), ("W2_l2", [64, 64]),
    ("W2_B4", [128, 128]),
    ("W3_0", [64, 320]), ("W3_12", [128, 128]), ("W3_l2", [64, 64]),
    ("W3_B4", [128, 128]),
    ("W4_B", [128, 13, D_OUT]), ("W4_A", [128, 5, D_OUT]), ("W4_0", [64, D_OUT]),
]


def _split_excess_waits(nc, max_waits=1):
    """This walrus build accepts only one sem-wait per instruction on
    some ops; hoist excess waits onto same-engine NoOps inserted before."""
    for f in nc.m.functions:
        for bb in f.blocks:
            newlist = []
            changed = False
            for ins in bb.instructions:
                si = ins.sync_info
                waits = list(si.on_wait) if (si and si.on_wait) else []
                if len(waits) > max_waits:
                    extras, keep = waits[:-max_waits], waits[-max_waits:]
                    for k in range(0, len(extras), max_waits):
                        nop = mybir.InstNoOp(
                            name=f"{ins.name}_waitnop{k}", ins=[], outs=[],
                            engine=ins.engine)
                        nop.sync_info = mybir.SyncInfo(
                            on_wait=extras[k:k + max_waits], on_update=[])
                        nc.register_instruction(nop)
                        newlist.append(nop)
                    ins.sync_info = mybir.SyncInfo(
                        on_wait=keep,
                        on_update=list(si.on_update) if si.on_update else [])
                    changed = True
                newlist.append(ins)
            if changed:
                bb.instructions[:] = newlist
    return nc


def _build_program():
    nc = bass.Bass("TRN2", target_bir_lowering=False, debug=False)

    vt = nc.dram_tensor("vt", [NT, 128, 30, BT], BF16, kind="ExternalInput").ap()
    wd = {}
    for name, shape in _WSPECS:
        wd[name] = nc.dram_tensor(name, shape, BF16, kind="ExternalInput").ap()
    out49 = nc.dram_tensor("out49", [D_OUT, BC], FP32, kind="ExternalOutput").ap()

    with tile.TileContext(nc) as tc:
        with ExitStack() as ctx:
            _emit(ctx, tc, nc, vt, wd, out49)

    _split_excess_waits(nc)
    return nc


def _emit(ctx, tc, nc, vt, wd, out49):
    mm = nc.tensor.matmul
    Sig = mybir.ActivationFunctionType.Sigmoid

    wpool = ctx.enter_context(tc.tile_pool(name="weights", bufs=1))
    vpool = ctx.enter_context(tc.tile_pool(name="vtiles", bufs=2))
    hpool = ctx.enter_context(tc.tile_pool(name="htiles", bufs=2))
    gpool = ctx.enter_context(tc.tile_pool(name="gates", bufs=2))
    opool = ctx.enter_context(tc.tile_pool(name="outs", bufs=2))
    zpool = ctx.enter_context(tc.tile_pool(name="zb", bufs=2, space="PSUM"))
    z4pool = ctx.enter_context(tc.tile_pool(name="z4", bufs=2, space="PSUM"))

    W = {}
    for name, _ in _WSPECS:
        t = wpool.tile(list(wd[name].shape), BF16, tag=name)
        nc.sync.dma_start(out=t, in_=wd[name])
        W[name] = t

    # valid-partition prefix of the B z-tile per m (layout l6|l5|l4|l3)
    def bphi(m):
        return 128 if m < 7 else (96 if m < 9 else (64 if m < 11 else 32))

    for t in range(NT):
        vtile = vpool.tile([128, 30, BT], BF16, tag="vt")
        nc.sync.dma_start(out=vtile, in_=vt[t])

        x = lambda g: vtile[:, g, :]
        xh = lambda g, h: vtile[64 * h:64 * (h + 1), g, :]

        def gates_from_z0(z0):
            h0 = hpool.tile([64, BT], BF16, tag="h0")
            gA = gpool.tile([128, BT], BF16, tag="gA")
            gB = gpool.tile([128, BT], BF16, tag="gB")
            sig = gpool.tile([64, BT], BF16, tag="sig")
            nc.scalar.activation(sig, z0[0:64, 0, :], Sig)
            nc.vector.tensor_mul(h0, z0[0:64, 0, :], sig)
            nc.scalar.activation(gA, z0[:, 1, :], Sig)
            nc.scalar.activation(gB, z0[:, 2, :], Sig)
            return h0, gA, gB

        def gate_A(zA1, zA2, gA):
            hA = hpool.tile([128, 5, BT], BF16, tag="hA")
            nc.vector.tensor_mul(
                hA[:, 0:3, :], zA1,
                gA.unsqueeze(1).broadcast_to([128, 3, BT]))
            nc.vector.tensor_mul(
                hA[0:64, 3:5, :], zA2[0:64, 0:2, :],
                gA[0:64, :].unsqueeze(1).broadcast_to([64, 2, BT]))
            return hA

        # ---------------- layer 1 ----------------
        z0 = zpool.tile([128, 3, BT], FP32, tag="zb")
        for s, (wk, g) in enumerate(((W["W1_0a"], 0), (W["W1_0b"], 1))):
            st, sp = (s == 0), (s == 1)
            mm(z0[0:64, 0, :], wk[:, 0:64], x(g), start=st, stop=sp)
            mm(z0[:, 1, :], wk[:, 64:192], x(g), start=st, stop=sp)
            mm(z0[:, 2, :], wk[:, 192:320], x(g), start=st, stop=sp)
        h0, gA, gB = gates_from_z0(z0)

        zA1 = zpool.tile([128, 3, BT], FP32, tag="zb")
        for m in range(3):
            mm(zA1[0:64, m, :], W["W1_l2"], x(5 + m), start=True, stop=True,
               tile_position=(0, 0))
            mm(zA1[64:128, m, :], W["W1_l1"], x(2 + m), start=True, stop=True,
               tile_position=(0, 64))
        zA2 = zpool.tile([128, 3, BT], FP32, tag="zb")
        for j, m in enumerate((3, 4)):
            mm(zA2[0:64, j, :], W["W1_l2"], x(5 + m), start=True, stop=True,
               tile_position=(0, 0))
        hA = gate_A(zA1, zA2, gA)

        hB = hpool.tile([128, 13, BT], BF16, tag="hB")
        for mlo in (0, 3, 6, 9, 12):
            nm = min(3, 13 - mlo)
            zB = zpool.tile([128, 3, BT], FP32, tag="zb")
            phis = []
            for j in range(nm):
                m = mlo + j
                if m < 7:
                    mm(zB[0:64, j, :], W["W1_56"], x(18 + m), start=True,
                       stop=True, tile_position=(0, 0))
                    mm(zB[64:128, j, :], W["W1_34"], x(10 + m), start=True,
                       stop=True, tile_position=(0, 64))
                elif m < 9:
                    h = m - 7
                    mm(zB[0:64, j, :], W["W1_56"], x(18 + m), start=True,
                       stop=True, tile_position=(0, 0))
                    mm(zB[64:96, j, :], W["W1_l4"][64 * h:64 * (h + 1), :],
                       xh(17, h), start=True, stop=True,
                       tile_position=(64 * h, 64))
                elif m < 11:
                    mm(zB[0:64, j, :], W["W1_56"], x(18 + m), start=True,
                       stop=True, tile_position=(0, 0))
                else:
                    h = m - 11
                    mm(zB[0:32, j, :], W["W1_l6"][64 * h:64 * (h + 1), :],
                       xh(29, h), start=True, stop=True,
                       tile_position=(64 * h, 0))
                phis.append(bphi(m))
            j = 0
            while j < nm:
                k = j
                while k < nm and phis[k] == phis[j]:
                    k += 1
                phi = phis[j]
                nc.vector.tensor_mul(
                    hB[0:phi, mlo + j:mlo + k, :], zB[0:phi, j:k, :],
                    gB[0:phi, :].unsqueeze(1).broadcast_to([phi, k - j, BT]))
                j = k

        # ---------------- layers 2, 3 ----------------
        for ln in ("W2", "W3"):
            w0, w12, wl2, wb4 = W[ln + "_0"], W[ln + "_12"], W[ln + "_l2"], W[ln + "_B4"]
            z0 = zpool.tile([128, 3, BT], FP32, tag="zb")
            mm(z0[0:64, 0, :], w0[:, 0:64], h0, start=True, stop=True)
            mm(z0[:, 1, :], w0[:, 64:192], h0, start=True, stop=True)
            mm(z0[:, 2, :], w0[:, 192:320], h0, start=True, stop=True)
            nh0, gA, gB = gates_from_z0(z0)

            zA1 = zpool.tile([128, 3, BT], FP32, tag="zb")
            for m in range(3):
                mm(zA1[:, m, :], w12, hA[:, m, :], start=True, stop=True,
                   tile_position=(0, 0))
            zA2 = zpool.tile([128, 3, BT], FP32, tag="zb")
            for j, m in enumerate((3, 4)):
                mm(zA2[0:64, j, :], wl2, hA[0:64, m, :], start=True, stop=True,
                   tile_position=(0, 0))
            nhA = gate_A(zA1, zA2, gA)

            nhB = hpool.tile([128, 13, BT], BF16, tag="hB")
            for mlo in (0, 3, 6, 9, 12):
                nm = min(3, 13 - mlo)
                zB = zpool.tile([128, 3, BT], FP32, tag="zb")
                phis = []
                for j in range(nm):
                    m = mlo + j
                    kp = bphi(m)
                    mm(zB[0:kp, j, :], wb4[0:kp, 0:kp], hB[0:kp, m, :],
                       start=True, stop=True, tile_position=(0, 0))
                    phis.append(kp)
                j = 0
                while j < nm:
                    k = j
                    while k < nm and phis[k] == phis[j]:
                        k += 1
                    phi = phis[j]
                    nc.vector.tensor_mul(
                        nhB[0:phi, mlo + j:mlo + k, :], zB[0:phi, j:k, :],
                        gB[0:phi, :].unsqueeze(1).broadcast_to([phi, k - j, BT]))
                    j = k

            h0, hA, hB = nh0, nhA, nhB

        # ---------------- layer 4 ----------------
        z4 = z4pool.tile([D_OUT, BT], FP32, tag="z4")
        mm(z4, W["W4_0"], h0, start=True, stop=False, tile_position=(0, 0))
        for m in range(5):
            kp = 128 if m < 3 else 64
            mm(z4, W["W4_A"][0:kp, m, :], hA[0:kp, m, :], start=False,
               stop=False, tile_position=(0, 0))
        for m in range(13):
            kp = bphi(m)
            mm(z4, W["W4_B"][0:kp, m, :], hB[0:kp, m, :], start=False,
               stop=(m == 12), tile_position=(0, 0))

        z4sb = opool.tile([D_OUT, BT], FP32, tag="z4sb")
        nc.scalar.copy(out=z4sb, in_=z4)
        nc.sync.dma_start(out=out49[:, t * BT:(t + 1) * BT], in_=z4sb)


def _get_nc():
    if "nc" not in _BUILD:
        _BUILD["nc"] = _build_program()
    return _BUILD["nc"]


def kernel(v_raw, w1, w2, w3, w4):
    nc = _get_nc()
    wmap = _pack_weights(np.asarray(w1), np.asarray(w2), np.asarray(w3),
                         np.asarray(w4))
    v_raw = np.asarray(v_raw, dtype=np.float32)
    vP = v_raw[:, P_FEAT].astype(BF)                 # [B, 3840] feature-permuted
    in_maps = []
    for c in range(NCORES):
        sl = vP[c * BC:(c + 1) * BC]                 # [BC, 3840]
        vt = np.ascontiguousarray(
            sl.reshape(NT, BT, 30, 128).transpose(0, 3, 2, 1))
        m = dict(wmap)
        m["vt"] = vt
        in_maps.append(m)
    res = bass_utils.run_bass_kernel_spmd(nc, in_maps, core_ids=list(range(NCORES)))
    outs = [res.results[c]["out49"] for c in range(NCORES)]   # [49, BC] each
    full = np.concatenate([o.T for o in outs], axis=0)        # [B, 49]
    return np.ascontiguousarray(full).reshape(B_FULL, D_OUT, 1).astype(np.float32)
